# revision 5
# baseline (speedup 1.0000x reference)
"""Trainium2 Bass kernel for causal multi-head attention (B=4, T=2048, D=1024, H=16).

Sharding: tensor-parallel over heads. Each of the 8 NeuronCores owns 2 heads:
it computes Q/K/V projections for its head-slice over all tokens, runs causal
attention, then an AllToAll re-shards the attention output from head-sharded
to token-sharded so each core computes the final FC layer for its 1024-token
block with the full weight matrix. No reduction collective is needed.

All matmuls run as float32r (TF32-like, full PE rate at moving dim >= 256);
everything else stays fp32. Scores are computed transposed (S^T = K Q^T) so
softmax normalization lands on the PV matmul's free dim; the denominator is
obtained by augmenting V with a ones column, and its reciprocal is broadcast
across partitions with a selector matmul (partition-shifted DVE/DMA ops are
avoided entirely -- they were observed to misbehave on this stack).
"""
import sys

for _p in ("/opt/trn_rl_repo",):
    if _p not in sys.path:
        sys.path.insert(0, _p)

import numpy as np

import concourse.bass as bass
import concourse.mybir as mybir
import concourse.tile as tile
from concourse import bacc
from concourse.bass_utils import run_bass_kernel_spmd

f32 = mybir.dt.float32
f32r = mybir.dt.float32r
EXP = mybir.ActivationFunctionType.Exp

B, T, D, H, HD = 4, 2048, 1024, 16, 64
NCORES = 8
HPC = H // NCORES          # heads per core = 2
BT = B * T                 # 8192
CH = 512                   # token chunk (matmul moving dim)
NCH_B = T // CH            # 4 projection chunks per batch
QC = T // CH               # 4 query chunks per batch
NKV_B = T // 128           # 16 kv tiles of 128 per batch
ROWS = BT // NCORES        # 1024 output token rows per core
SCALE = 1.0 / 8.0          # 1/sqrt(HD)

_CACHE = {}


def _build(sim=False, no_collective=False):
    nc = bacc.Bacc("TRN2", target_bir_lowering=False, debug=False,
                   num_devices=1 if sim else NCORES)

    xT = nc.dram_tensor("xT", [D, BT], f32, kind="ExternalInput").ap()
    wqkv = nc.dram_tensor("wqkv", [D, 3 * 128], f32, kind="ExternalInput").ap()
    bqkv = nc.dram_tensor("bqkv", [1, 3 * 128], f32, kind="ExternalInput").ap()
    wfc_d = nc.dram_tensor("wfc", [D, 128], f32, kind="ExternalInput").ap()
    bfc_d = nc.dram_tensor("bfc", [1, 128], f32, kind="ExternalInput").ap()
    hm_d = nc.dram_tensor("hm", [128, 896], f32, kind="ExternalInput").ap()
    zl_d = nc.dram_tensor("zl", [65, 64], f32, kind="ExternalInput").ap()
    id_d = nc.dram_tensor("ident", [128, 128], f32, kind="ExternalInput").ap()
    ones_d = nc.dram_tensor("ones", [128, CH], f32, kind="ExternalInput").ap()
    zer_d = nc.dram_tensor("zer", [65, CH], f32, kind="ExternalInput").ap()
    outT = nc.dram_tensor("outT", [128, BT], f32, kind="ExternalOutput").ap()

    with tile.TileContext(nc) as tc:
        with tc.tile_pool(name="const", bufs=1) as cst, \
             tc.tile_pool(name="dram", bufs=1, space="DRAM") as dpool:

            # ---- constants (host-provided) ----
            ones_r = cst.tile([128, CH], f32r)
            nc.sync.dma_start(ones_r[:], ones_d[:].bitcast(f32r))
            ident = cst.tile([128, 128], f32r)
            nc.sync.dma_start(ident[:], id_d[:].bitcast(f32r))
            hm = cst.tile([128, 896], f32)
            nc.sync.dma_start(hm[:], hm_d[:])
            zl = cst.tile([65, 64], f32r)       # selector: row 64 = 1, rest 0
            nc.sync.dma_start(zl[:], zl_d[:].bitcast(f32r))
            bias_q = cst.tile([1, 3 * 128], f32r)
            nc.sync.dma_start(bias_q[:], bqkv[:].bitcast(f32r))
            bias_f = cst.tile([1, 128], f32r)
            nc.sync.dma_start(bias_f[:], bfc_d[:].bitcast(f32r))
            # reciprocal staging tiles (rows 0..63 stay zero forever)
            rc = []
            for h in range(HPC):
                t = cst.tile([65, CH], f32r, name=f"recip{h}")
                nc.sync.dma_start(t[:], zer_d[:].bitcast(f32r))
                rc.append(t)
            rtmp = cst.tile([65, CH], f32)      # fp32 reciprocal staging

            # attention output, head-sharded: per local head [64, BT]
            attn = [cst.tile([64, BT], f32r, name=f"attn{h}")
                    for h in range(HPC)]

            # qkv weights: 8 d-tiles of [128, 384] = [q128 | k128 | v128]
            wq = cst.tile([128, 8 * 384], f32r)
            for d in range(8):
                nc.sync.dma_start(wq[:, d * 384:(d + 1) * 384],
                                  wqkv[d * 128:(d + 1) * 128, :].bitcast(f32r))

            # ---- per-batch QKV projection + attention ----
            with tc.tile_pool(name="work", bufs=1) as wk, \
                 tc.tile_pool(name="ps", bufs=1, space="PSUM") as ps:
                for b in range(B):
                    t0 = b * T
                    qt = wk.tile([128, T], f32r, tag="qt", bufs=2, name=f"qt{b}")
                    kt = wk.tile([128, T], f32r, tag="kt", bufs=2, name=f"kt{b}")
                    vsb = wk.tile([128, NKV_B * 130], f32r, tag="vsb", bufs=2,
                                  name=f"vsb{b}")
                    # ones columns (denominator) for all 16 kv tiles of batch b
                    v3 = vsb.rearrange("p (t c) -> p t c", c=130)
                    src1 = ones_d[:, 0:NKV_B].rearrange("p (t c) -> p t c", c=1)
                    nc.sync.dma_start(v3[:, :, 64:65], src1.bitcast(f32r))
                    nc.sync.dma_start(v3[:, :, 129:130], src1.bitcast(f32r))

                    for ch in range(NCH_B):
                        c0 = t0 + ch * CH
                        xt = wk.tile([128, 8 * CH], f32r, tag="xt", bufs=2,
                                     name=f"xt{b}_{ch}")
                        for d in range(8):
                            nc.sync.dma_start(
                                xt[:, d * CH:(d + 1) * CH],
                                xT[d * 128:(d + 1) * 128,
                                   c0:c0 + CH].bitcast(f32r))
                        cs = ch * CH
                        # Q^T chunk
                        psq = ps.tile([128, CH], f32, tag="psq", bufs=1,
                                      name=f"psq{b}_{ch}")
                        for d in range(8):
                            nc.tensor.matmul(psq[:],
                                             wq[:, d * 384:d * 384 + 128],
                                             xt[:, d * CH:(d + 1) * CH],
                                             start=(d == 0), stop=False)
                        nc.tensor.matmul(psq[:], bias_q[0:1, 0:128],
                                         ones_r[0:1, :], start=False, stop=True)
                        nc.vector.tensor_copy(qt[:, cs:cs + CH], psq[:])
                        # K^T chunk
                        psk = ps.tile([128, CH], f32, tag="psk", bufs=1,
                                      name=f"psk{b}_{ch}")
                        for d in range(8):
                            nc.tensor.matmul(psk[:],
                                             wq[:, d * 384 + 128:d * 384 + 256],
                                             xt[:, d * CH:(d + 1) * CH],
                                             start=(d == 0), stop=False)
                        nc.tensor.matmul(psk[:], bias_q[0:1, 128:256],
                                         ones_r[0:1, :], start=False, stop=True)
                        nc.vector.tensor_copy(kt[:, cs:cs + CH], psk[:])
                        # V^T chunk -> transpose to token-major V
                        psv = ps.tile([128, CH], f32, tag="psv", bufs=1,
                                      name=f"psv{b}_{ch}")
                        for d in range(8):
                            nc.tensor.matmul(psv[:],
                                             wq[:, d * 384 + 256:d * 384 + 384],
                                             xt[:, d * CH:(d + 1) * CH],
                                             start=(d == 0), stop=False)
                        nc.tensor.matmul(psv[:], bias_q[0:1, 256:384],
                                         ones_r[0:1, :], start=False, stop=True)
                        vts = wk.tile([128, CH], f32r, tag="vts", bufs=2,
                                      name=f"vts{b}_{ch}")
                        nc.vector.tensor_copy(vts[:], psv[:])
                        for s in range(CH // 128):
                            kvt = ch * 4 + s    # kv tile idx within batch
                            pst = ps.tile([128, 128], f32, tag="pst", bufs=1,
                                          name=f"pst{b}_{ch}_{s}")
                            nc.tensor.transpose(pst[:].bitcast(f32r),
                                                vts[:, s * 128:(s + 1) * 128],
                                                ident[:])
                            base = kvt * 130
                            nc.vector.tensor_copy(vsb[:, base:base + 64],
                                                  pst[:, 0:64])
                            nc.vector.tensor_copy(vsb[:, base + 65:base + 129],
                                                  pst[:, 64:128])

                    # ---- causal attention for batch b ----
                    for qc in range(QC):
                        g0 = t0 + qc * CH
                        nkv = 4 * (qc + 1)
                        pv = [ps.tile([128, CH], f32, tag=f"pv{h}", bufs=1,
                                      name=f"pv{h}_{b}_{qc}")
                              for h in range(HPC)]
                        for ki in range(nkv):
                            diag = ki - 4 * qc  # >=0 on diagonal block tiles
                            for h in range(HPC):
                                st = ps.tile([128, CH], f32, tag="s", bufs=2,
                                             name=f"s{h}_{b}_{qc}_{ki}")
                                pt = wk.tile([128, CH], f32r, tag="p", bufs=4,
                                             name=f"p{h}_{b}_{qc}_{ki}")
                                nc.tensor.matmul(
                                    st[:],
                                    kt[64 * h:64 * h + 64,
                                       ki * 128:(ki + 1) * 128],
                                    qt[64 * h:64 * h + 64,
                                       qc * CH:(qc + 1) * CH],
                                    start=True, stop=True,
                                    tile_position=(64 * h, 0))
                                nc.scalar.activation(pt[:], st[:], EXP,
                                                     scale=SCALE)
                                if diag >= 0:
                                    off = 384 - 128 * diag
                                    nc.vector.tensor_mul(pt[:], pt[:],
                                                         hm[:, off:off + CH])
                                vb = ki * 130 + 65 * h
                                nc.tensor.matmul(pv[h][0:65, :],
                                                 vsb[:, vb:vb + 65],
                                                 pt[:],
                                                 start=(ki == 0),
                                                 stop=(ki == nkv - 1))
                        # normalize: reciprocal of denom row, broadcast, mul
                        for h in range(HPC):
                            nc.vector.reciprocal(rtmp[64:65, :],
                                                 pv[h][64:65, :])
                            nc.vector.tensor_copy(rc[h][64:65, :],
                                                  rtmp[64:65, :])
                            bc = ps.tile([64, CH], f32, tag="s", bufs=2,
                                         name=f"bc{h}_{b}_{qc}")
                            nc.tensor.matmul(bc[:], zl[:], rc[h][:],
                                             start=True, stop=True)
                            rb = wk.tile([64, CH], f32, tag="rb", bufs=2,
                                         name=f"rb{h}_{b}_{qc}")
                            nc.vector.tensor_copy(rb[:], bc[:])
                            nc.vector.tensor_mul(attn[h][:, g0:g0 + CH],
                                                 pv[h][0:64, :], rb[:])

            # ---- AllGather attention output (head-sharded -> replicated) ----
            ag_in = dpool.tile([128, BT], f32)
            ag_out = dpool.tile([NCORES * 128, BT], f32)
            for h in range(HPC):
                nc.sync.dma_start(ag_in[64 * h:64 * h + 64, :],
                                  attn[h][:].bitcast(f32))
            if sim or no_collective:
                nc.sync.dma_start(ag_out[0:128, :], ag_in[:])
            else:
                nc.gpsimd.collective_compute(
                    "AllGather", mybir.AluOpType.bypass,
                    replica_groups=[list(range(NCORES))],
                    ins=[ag_in.opt()], outs=[ag_out.opt()])

            # ---- final FC: this core computes its 128 output features for
            # all tokens (weight slice is per-core host input) ----
            with tc.tile_pool(name="fcp", bufs=1) as fcp, \
                 tc.tile_pool(name="psc", bufs=1, space="PSUM") as psc:
                wfc = fcp.tile([128, 8 * 128], f32r)
                for d in range(8):
                    nc.sync.dma_start(
                        wfc[:, d * 128:(d + 1) * 128],
                        wfc_d[d * 128:(d + 1) * 128, :].bitcast(f32r))
                for oc in range(BT // CH):
                    fci = fcp.tile([128, 8 * CH], f32r, tag="fci", bufs=3,
                                   name=f"fci{oc}")
                    for d in range(8):
                        nc.sync.dma_start(
                            fci[:, d * CH:(d + 1) * CH],
                            ag_out[d * 128:(d + 1) * 128,
                                   oc * CH:(oc + 1) * CH].bitcast(f32r))
                    pfc = psc.tile([128, CH], f32, tag="fc", bufs=4,
                                   name=f"pfc{oc}")
                    for d in range(8):
                        nc.tensor.matmul(pfc[:],
                                         wfc[:, d * 128:(d + 1) * 128],
                                         fci[:, d * CH:(d + 1) * CH],
                                         start=(d == 0), stop=False)
                    nc.tensor.matmul(pfc[:], bias_f[0:1, :],
                                     ones_r[0:1, :], start=False, stop=True)
                    ost = fcp.tile([128, CH], f32, tag="ost", bufs=4,
                                   name=f"ost{oc}")
                    nc.vector.tensor_copy(ost[:], pfc[:])
                    nc.sync.dma_start(outT[:, oc * CH:(oc + 1) * CH], ost[:])

    nc.compile()
    return nc


def _host_inputs(x, W_qkv, b_qkv, W_fc, b_fc):
    x = np.asarray(x, dtype=np.float32)
    W_qkv = np.asarray(W_qkv, dtype=np.float32)
    b_qkv = np.asarray(b_qkv, dtype=np.float32)
    W_fc = np.asarray(W_fc, dtype=np.float32)
    b_fc = np.asarray(b_fc, dtype=np.float32)

    xT = np.ascontiguousarray(x.reshape(BT, D).T)
    hm = (np.arange(128)[:, None]
          <= np.arange(896)[None, :] - 384).astype(np.float32)
    zl = np.zeros((65, 64), np.float32)
    zl[64, :] = 1.0
    ident = np.eye(128, dtype=np.float32)
    ones = np.ones((128, CH), np.float32)
    zer = np.zeros((65, CH), np.float32)
    in_maps = []
    for c in range(NCORES):
        f0 = c * (HPC * HD)  # 128*c
        wfc_c = np.ascontiguousarray(W_fc[:, f0:f0 + 128])
        bfc_c = np.ascontiguousarray(b_fc[None, f0:f0 + 128])
        wq_c = np.ascontiguousarray(np.concatenate(
            [W_qkv[:, p * D + f0: p * D + f0 + 128] for p in range(3)], axis=1))
        bq_c = np.ascontiguousarray(np.concatenate(
            [b_qkv[p * D + f0: p * D + f0 + 128] for p in range(3)])[None, :])
        in_maps.append({
            "xT": xT, "wqkv": wq_c, "bqkv": bq_c, "wfc": wfc_c, "bfc": bfc_c,
            "hm": hm, "zl": zl, "ident": ident, "ones": ones, "zer": zer,
        })
    return in_maps


def _get_nc():
    if "nc" not in _CACHE:
        _CACHE["nc"] = _build()
    return _CACHE["nc"]


def _assemble(results):
    blocks = [results[c]["outT"] for c in range(NCORES)]
    full = np.concatenate(blocks, axis=0)          # [D, BT], feature-major
    return np.ascontiguousarray(full.T).reshape(B, T, D).astype(np.float32)


def kernel(x, W_qkv, b_qkv, W_fc, b_fc):
    nc = _get_nc()
    in_maps = _host_inputs(x, W_qkv, b_qkv, W_fc, b_fc)
    res = run_bass_kernel_spmd(nc, in_maps, list(range(NCORES)))
    return _assemble(res.results)
